# revision 8
# baseline (speedup 1.0000x reference)
"""Trainium2 Bass kernel for nn_KeyDecider: per-(b,ch) spatial softmax +
soft-argmax + confidence, batch-sharded across 8 NeuronCores.

Input : x [64, 34, 256, 256] f32
Output: [64, 17, 3] f32  (co_x, co_y, confidence)

Math (per b, c<17):  w = softmax(x[b,c].ravel());  v = x[b,c+17].ravel()
  ki = round(sum(w*p));  out = [ki%256, ki//256, sum(w*v)]
exp() needs no max-subtraction (inputs are randn, |x|<6), so a single
pass over HBM suffices.

DMA layout (the key optimization): batch-strided access patterns run at
~174 GB/s on this part, while fully contiguous loads reach ~352 GB/s
(the HBM-per-core limit).  One channel is 64K elems = 128 x 512, so a
contiguous 2MB load [128, 4096] covers exactly 8 channels of one batch
with channel boundaries aligned to partition boundaries (16 partitions
per channel, 4096 contiguous elems per partition).  Per (batch, h/v)
the core issues two 2MB loads (ch 0-7, ch 8-15) plus one 256KB load
(ch 16, [128, 512] = 512-elem blocks).  The matching v-tile lives
exactly 17*N elements later with identical layout, so e*v elementwise
ops align.

Per tile the device computes three per-partition sums into stats cols:
  s0 = sum(e)         ACT exp with fused accum
  s1 = sum(e*j_local) DVE mult + ACT identity with fused accum
  s2 = sum(e*v)       DVE mult + (DVE reduce | ACT identity accum)
s2 columns are split across two stats tiles (one per engine) to avoid
cross-engine write serialization.  The host combines partials in f64,
folding in each partition-row's base offset (offs * s0) exactly.
(tensor_tensor_reduce would fuse mult+reduce but hard-crashes the NEFF
on this toolchain - verified with a minimal repro.)
"""

import sys

for _p in ("/root/.axon_site/_ro/trn_rl_repo", "/opt/trn_rl_repo"):
    if _p not in sys.path:
        sys.path.insert(0, _p)
# /opt ends up first: the _ro copy has newer modules that mismatch
# sibling packages.

import numpy as np

B, C, K, N = 64, 34, 17, 256 * 256
W = H = 256
IMG_W = IMG_H = 256.0
NCORES = 8
BPC = B // NCORES          # batches per core
BIGW = 4096                # big-tile free width (8 channels, 2MB)
SMLW = 512                 # small-tile free width (1 channel, 256KB)
TPB = 3                    # tiles per (batch, h/v): 8ch + 8ch + 1ch
COLS = BPC * TPB           # stats columns per core

# s2 reduction engine per tile index: True -> ACT identity+accum
_S2_ON_ACT = lambda idx: idx % 3 != 0

_cache = {}


def _emit(nc, bass, tile, mybir, tc, x_ap, s_outs, loop_k):
    f32 = mybir.dt.float32
    with (
        tc.tile_pool(name="hp", bufs=3) as hp,
        tc.tile_pool(name="vp", bufs=2) as vp,
        tc.tile_pool(name="ep", bufs=2) as ep,
        tc.tile_pool(name="s1p", bufs=2) as s1p,
        tc.tile_pool(name="s2p", bufs=2) as s2p,
        tc.tile_pool(name="const", bufs=1) as const,
        tc.tile_pool(name="stats", bufs=1) as stats,
    ):
        # iota scratch borrows an s1p slot (released after the copy)
        pb_i = s1p.tile([128, BIGW], mybir.dt.int32, tag="s1")
        nc.gpsimd.iota(pb_i[:], pattern=[[1, BIGW]], base=0,
                       channel_multiplier=0)
        pb = const.tile([128, BIGW], f32)
        nc.vector.tensor_copy(pb[:], pb_i[:])

        s0_t = stats.tile([128, COLS], f32)
        s1_t = stats.tile([128, COLS], f32)
        s2_t = stats.tile([128, COLS], f32)   # DVE-reduced columns
        s2a_t = stats.tile([128, COLS], f32)  # ACT-accumulated columns
        nc.vector.memset(s2_t[:], 0.0)
        nc.vector.memset(s2a_t[:], 0.0)

        def body():
            idx = 0
            for b in range(BPC):
                for k in range(TPB):
                    col = b * TPB + k
                    w = BIGW if k < 2 else SMLW
                    off = b * C * N + k * 128 * BIGW
                    src_h = bass.AP(tensor=x_ap.tensor, offset=off,
                                    ap=[[w, 128], [1, w]])
                    src_v = bass.AP(tensor=x_ap.tensor, offset=off + K * N,
                                    ap=[[w, 128], [1, w]])
                    ht = hp.tile([128, w], f32, tag="h")
                    nc.sync.dma_start(out=ht[:], in_=src_h)
                    vt = vp.tile([128, w], f32, tag="v")
                    nc.sync.dma_start(out=vt[:], in_=src_v)

                    et = ep.tile([128, w], f32, tag="e")
                    nc.scalar.activation(
                        et[:], ht[:], mybir.ActivationFunctionType.Exp,
                        accum_out=s0_t[:, col:col + 1],
                    )
                    sc1 = s1p.tile([128, w], f32, tag="s1")
                    nc.vector.tensor_tensor(
                        out=sc1[:], in0=et[:], in1=pb[:, :w],
                        op=mybir.AluOpType.mult,
                    )
                    nc.scalar.activation(
                        sc1[:], sc1[:], mybir.ActivationFunctionType.Identity,
                        accum_out=s1_t[:, col:col + 1],
                    )
                    sc2 = s2p.tile([128, w], f32, tag="s2")
                    nc.vector.tensor_tensor(
                        out=sc2[:], in0=et[:], in1=vt[:],
                        op=mybir.AluOpType.mult,
                    )
                    if _S2_ON_ACT(idx):
                        nc.scalar.activation(
                            sc2[:], sc2[:],
                            mybir.ActivationFunctionType.Identity,
                            accum_out=s2a_t[:, col:col + 1],
                        )
                    else:
                        nc.vector.reduce_sum(
                            s2_t[:, col:col + 1], sc2[:],
                            axis=mybir.AxisListType.X,
                        )
                    idx += 1

        if loop_k == 1:
            body()
        else:
            with tc.For_i(0, loop_k, 1):
                body()

        nc.sync.dma_start(out=s_outs[0][:], in_=s0_t[:])
        nc.sync.dma_start(out=s_outs[1][:], in_=s1_t[:])
        nc.sync.dma_start(out=s_outs[2][:], in_=s2_t[:])
        nc.sync.dma_start(out=s_outs[3][:], in_=s2a_t[:])


def _build(loop_k: int = 1, scratch: bool = False):
    import concourse.bass as bass
    import concourse.bacc as bacc
    import concourse.tile as tile
    from concourse import mybir

    f32 = mybir.dt.float32
    nc = bacc.Bacc("TRN2", target_bir_lowering=False, debug=False)
    if scratch:
        nc.declare_dram_parameter("x", [128, 16], f32, isOutput=False)
    else:
        x_d = nc.declare_dram_parameter("x", [BPC, C, N], f32, isOutput=False)
    s_outs = [
        nc.declare_dram_parameter(n, [128, COLS], f32, isOutput=True)
        for n in ("s0", "s1", "s2", "s2a")
    ]
    with tile.TileContext(nc) as tc:
        if scratch:
            with tc.tile_pool(name="dram", bufs=1, space="DRAM") as dp:
                xd = dp.tile([BPC, C, N], f32)
                _emit(nc, bass, tile, mybir, tc, xd[:], s_outs, loop_k)
        else:
            _emit(nc, bass, tile, mybir, tc, x_d[:], s_outs, loop_k)
    nc.compile()
    return nc


def _run_device(x: np.ndarray):
    """Run the device part; returns per-core stats arrays (list of dicts)."""
    from concourse.bass_utils import run_bass_kernel_spmd

    if "nc" not in _cache:
        _cache["nc"] = _build()
    nc = _cache["nc"]
    in_maps = [
        {"x": np.ascontiguousarray(x[i * BPC:(i + 1) * BPC]).reshape(BPC, C, N)}
        for i in range(NCORES)
    ]
    return run_bass_kernel_spmd(nc, in_maps, list(range(NCORES)))


def _finish(results) -> np.ndarray:
    """Combine per-core partials (f64) into the [64,17,3] output.

    Stats column (b, k): k<2 -> big tile covering channels k*8..k*8+7,
    partition p = (channel k*8 + p//16, 4096-elem block p%16).
    k=2 -> small tile, channel 16, partition p = 512-elem block p.
    """
    out = np.empty((B, K, 3), np.float32)
    sel_act = np.array([_S2_ON_ACT(i) for i in range(COLS)])
    for i in range(NCORES):
        r = results[i]
        S2m = np.where(sel_act[None, :], r["s2a"], r["s2"])
        S0 = r["s0"].astype(np.float64)
        S1 = r["s1"].astype(np.float64)
        S2 = S2m.astype(np.float64)
        s0 = np.zeros((BPC, K)); s1 = np.zeros((BPC, K)); s2 = np.zeros((BPC, K))
        for b in range(BPC):
            for k in range(2):
                col = b * TPB + k
                # [128] -> [8 channels, 16 blocks of 4096]
                a0 = S0[:, col].reshape(8, 16)
                a1 = S1[:, col].reshape(8, 16)
                a2 = S2[:, col].reshape(8, 16)
                offs = (np.arange(16) * 4096).astype(np.float64)[None, :]
                s0[b, k * 8:(k + 1) * 8] = a0.sum(1)
                s1[b, k * 8:(k + 1) * 8] = (a1 + offs * a0).sum(1)
                s2[b, k * 8:(k + 1) * 8] = a2.sum(1)
            col = b * TPB + 2
            offs = (np.arange(128) * 512).astype(np.float64)
            s0[b, 16] = S0[:, col].sum()
            s1[b, 16] = (S1[:, col] + offs * S0[:, col]).sum()
            s2[b, 16] = S2[:, col].sum()
        ki = np.round(s1 / s0)
        co_x = np.mod(ki, W) / W * IMG_W
        co_y = np.floor(ki / W) / H * IMG_H
        vi = s2 / s0
        out[i * BPC:(i + 1) * BPC] = np.stack(
            [co_x, co_y, vi], axis=-1).astype(np.float32)
    return out


def kernel(x: np.ndarray) -> np.ndarray:
    res = _run_device(x)
    return _finish(res.results)
